# revision 1
# baseline (speedup 1.0000x reference)
"""Trainium2 Bass kernel for nn_DecoderRNN (Bahdanau-attention GRU decoder).

v2: pure data parallel over batch (128 -> 16 per core, 8 cores), bf16 matmuls
with f32 PSUM accumulation.

Per step (engines):
  hq = h @ Wh^T + bh         PE: 16 N=512 streaming MMs + 8 transposes to
                             packed [128,(j,b)] layout (bias folded by DVE)
  gh = h @ W_hh^T            PE: 48 N=512 MMs interleaved under the attention
                             window, evacuated to SBUF bf16 (DVE/ACT)
  X = tanh(proj + hq)        DMA streams proj rows from HBM; DVE/GPSIMD
                             broadcast-add in place; ACT tanh
  scores = v . X             PE: 7 chunk-PSUMs accumulated over 8 h-tiles
  w = softmax(scores)        ACT exp+accum, DVE; w scattered into a padded
                             block-diagonal lhsT via 2 PE transposes + 2 DMAs
  ctx = sum_n w*feat         PE: 64 N=512 MMs; transposed to packed layout
  gi = ctx @ Wx^T            PE: 48 N=512 MMs, Wx streamed from HBM
  GRU elementwise            [16,*] layout split across DVE/GPSIMD/ACT
cnn_proj = feat @ Wc^T + bc computed on device at startup -> HBM, re-streamed
each step. Classifier (h_t @ Wcls^T) at the end from h history spilled to HBM.
"""
import os
import sys

sys.path.insert(0, "/opt/trn_rl_repo")

import numpy as np
import ml_dtypes

import concourse.bass as bass
import concourse.tile as tile
from concourse import mybir
from concourse.bass_utils import run_bass_kernel_spmd
from concourse.masks import make_identity

F32 = mybir.dt.float32
BF16 = mybir.dt.bfloat16
bf = ml_dtypes.bfloat16
AL = mybir.AluOpType
AF = mybir.ActivationFunctionType

NCORES = 8
B = 16            # local batch per core
N = 196           # attention positions
H = 1024          # hidden
E = 512           # embed dim
G = 3 * H         # gate width
T = int(os.environ.get("DECODER_STEPS", "17"))
C = 1000          # classes
BN = B * N        # 3136
HBN = BN // 2     # half row 1568
KH = 8            # h k-tiles (1024/128)
KB = 32           # padded (b,n) k-tiles (16*256/128)
SCW = 448         # scores chunk width
NSC = 7           # scores chunks (7*448 = 3136)
SU = 392          # startup chunk width
CT = 8            # classifier m-tiles (1000 -> 7*128+104)
TB = T * B

_CACHE = {}


def _sc_pieces(c):
    """Batch-aligned pieces of scores chunk c: (src_lo, src_hi, b, d_lo)."""
    lo, hi = c * SCW, (c + 1) * SCW
    out = []
    b = lo // N
    while b * N < hi:
        s, e = max(lo, b * N), min(hi, (b + 1) * N)
        out.append((s - lo, e - lo, b, s - b * N))
        b += 1
    return out


def _split_waits(nc, keep=1):
    """This container's walrus build rejects >1 sem-wait per instruction
    (setupSyncWait: 'Too many sync wait commands'). Hoist all but one wait
    of every instruction onto single-wait NoOps on the same engine, placed
    immediately before it in program order."""
    nfix = 0
    for bb in nc.main_func.blocks:
        il = bb.instructions
        i = 0
        while i < len(il):
            ins = il[i]
            si = getattr(ins, 'sync_info', None)
            if si is not None and len(si.on_wait) > keep:
                waits = list(si.on_wait)
                for w_i, w in enumerate(waits[:-keep]):
                    nop = mybir.InstNoOp(name=f"{ins.name}-ws{w_i}", ins=[],
                                         outs=[])
                    nop.engine = ins.engine
                    nop.sync_info = mybir.SyncInfo(on_wait=[w], on_update=[])
                    il.insert(i, nop)
                    i += 1
                ins.sync_info = mybir.SyncInfo(on_wait=waits[-keep:],
                                               on_update=list(si.on_update))
                nfix += 1
            i += 1
    return nfix


def _build_program():
    nc = bass.Bass()

    featp_d = nc.declare_dram_parameter("featp", [KB, 128, H], BF16, isOutput=False)
    featT_d = nc.declare_dram_parameter("featT", [KH, 128, BN], BF16, isOutput=False)
    wcT_d = nc.declare_dram_parameter("wcT", [KH, 128, H], BF16, isOutput=False)
    wxT_d = nc.declare_dram_parameter("wxT", [KH, 128, G], BF16, isOutput=False)
    whhT_d = nc.declare_dram_parameter("whhT", [KH, 128, G], BF16, isOutput=False)
    whT_d = nc.declare_dram_parameter("whT", [KH, 128, H], BF16, isOutput=False)
    wclsT_d = nc.declare_dram_parameter("wclsT", [KH, 128, C], BF16, isOutput=False)
    vrep_d = nc.declare_dram_parameter("vrep", [KH, 128, B], BF16, isOutput=False)
    ge_d = nc.declare_dram_parameter("ge", [T, B, G], BF16, isOutput=False)
    h0b_d = nc.declare_dram_parameter("h0b", [B, H], F32, isOutput=False)
    hpk0_d = nc.declare_dram_parameter("hpk0", [128, 128], BF16, isOutput=False)
    bhpk_d = nc.declare_dram_parameter("bhpk", [128, 128], BF16, isOutput=False)
    bc_d = nc.declare_dram_parameter("bc", [1, H], BF16, isOutput=False)
    out_d = nc.declare_dram_parameter("out", [CT, 128, TB], F32, isOutput=True)

    projT_d = nc.dram_tensor("projT", [KH, 128, BN], BF16)
    hsd_d = nc.dram_tensor("hsd", [T, 128, 128], BF16)

    with tile.TileContext(nc) as tc:
        with tc.tile_pool(name="persist", bufs=1) as P1, \
             tc.tile_pool(name="state", bufs=2) as P2:

            # ---- persistent tensors
            feat_s = P1.tile([128, KB, H], BF16)
            for kb in range(KB):
                nc.sync.dma_start(feat_s[:, kb, :], featp_d[kb])
            whhT_s = P1.tile([128, KH, G], BF16)
            vrep_s = P1.tile([128, KH, B], BF16)
            for k in range(KH):
                nc.sync.dma_start(whhT_s[:, k, :], whhT_d[k])
                nc.sync.dma_start(vrep_s[:, k, :], vrep_d[k])
            ident16 = P1.tile([B, B], BF16)
            make_identity(nc, ident16)
            bhpk_s = P1.tile([128, 128], BF16)
            nc.sync.dma_start(bhpk_s, bhpk_d[:])
            wblk = P1.tile([128, 33 * B], BF16)
            nc.vector.memset(wblk, 0.0)
            w_s = P1.tile([B, N], BF16)

            h32 = P2.tile([B, H], F32, tag="h32")
            nc.sync.dma_start(h32, h0b_d[:])
            hpk = P2.tile([128, 128], BF16, tag="hpk")
            nc.sync.dma_start(hpk, hpk0_d[:])

            # ---- startup: cnn_proj = feat @ Wc^T + bc  -> projT_d (HBM)
            with tc.tile_pool(name="wcpool", bufs=1) as Pwc, \
                 tc.tile_pool(name="ftring", bufs=12) as Pft, \
                 tc.tile_pool(name="stage", bufs=4) as Pstage, \
                 tc.tile_pool(name="ps_start", bufs=3, space="PSUM") as PSs:
                wcT_s = Pwc.tile([128, KH, H], BF16)
                ones392 = Pwc.tile([1, SU], BF16)
                nc.vector.memset(ones392, 1.0)
                bc_s = Pwc.tile([1, H], BF16)
                nc.sync.dma_start(bc_s, bc_d[:])
                for k in range(KH):
                    nc.sync.dma_start(wcT_s[:, k, :], wcT_d[k])
                for cch in range(8):
                    sl = slice(cch * SU, (cch + 1) * SU)
                    fts = []
                    for k in range(KH):
                        ft = Pft.tile([128, SU], BF16, tag="ft",
                                      name=f"ft{cch}_{k}")
                        nc.sync.dma_start(ft, featT_d[k][:, sl])
                        fts.append(ft)
                    for m in range(KH):
                        ps = PSs.tile([128, SU], F32, tag="ps",
                                      name=f"ps{cch}_{m}")
                        nc.tensor.matmul(
                            ps, bc_s[0:1, m * 128:(m + 1) * 128], ones392,
                            start=True, stop=False)
                        for k in range(KH):
                            nc.tensor.matmul(
                                ps, wcT_s[:, k, m * 128:(m + 1) * 128], fts[k],
                                start=False, stop=(k == KH - 1))
                        st = Pstage.tile([128, SU], BF16, tag="st",
                                         name=f"st{cch}_{m}")
                        if m % 2 == 0:
                            nc.vector.tensor_copy(st, ps)
                        else:
                            nc.scalar.activation(st, ps, AF.Copy)
                        nc.sync.dma_start(projT_d[m][:, sl], st)

            # ---- decode steps
            with tc.tile_pool(name="projring", bufs=4) as Pstr, \
                 tc.tile_pool(name="xring", bufs=2) as Px, \
                 tc.tile_pool(name="whtring", bufs=2) as Pwht, \
                 tc.tile_pool(name="wxring", bufs=2) as Pwx, \
                 tc.tile_pool(name="gering", bufs=1) as Pge, \
                 tc.tile_pool(name="small", bufs=1) as Psm, \
                 tc.tile_pool(name="gt", bufs=2) as Pgt, \
                 tc.tile_pool(name="gf", bufs=2) as Pgf:
                for t in range(T):
                    ge_t = Pge.tile([B, G], BF16, tag="ge", name=f"ge{t}")
                    nc.sync.dma_start(ge_t, ge_d[t])

                    # ---- hq (packed via transposes, bias folded)
                    hqf = Psm.tile([B, H], BF16, tag="hqf", name=f"hqf{t}")
                    hq_sb = Psm.tile([128, 128], BF16, tag="hqsb",
                                     name=f"hqsb{t}", bufs=2)
                    with tc.tile_pool(name="psA", bufs=1, space="PSUM") as PA, \
                         tc.tile_pool(name="psT", bufs=2, space="PSUM") as PT:
                        pqs = [PA.tile([B, 512], F32, tag=f"hqp{c}",
                                       name=f"hqp{t}_{c}") for c in range(2)]
                        for k in range(KH):
                            wht_k = Pwht.tile([128, H], BF16, tag="wht",
                                              name=f"wht{t}_{k}")
                            nc.sync.dma_start(wht_k, whT_d[k])
                            for c in range(2):
                                nc.tensor.matmul(
                                    pqs[c], hpk[:, k * B:(k + 1) * B],
                                    wht_k[:, c * 512:(c + 1) * 512],
                                    start=(k == 0), stop=(k == KH - 1))
                        for c in range(2):
                            nc.vector.tensor_copy(
                                hqf[:, c * 512:(c + 1) * 512], pqs[c])
                        for m in range(KH):
                            tp = PT.tile([128, B], BF16, tag="tphq",
                                         name=f"tphq{t}_{m}")
                            nc.tensor.transpose(
                                tp, hqf[:, m * 128:(m + 1) * 128], ident16)
                            nc.vector.scalar_tensor_tensor(
                                out=hq_sb[:, m * B:(m + 1) * B], in0=tp,
                                scalar=1.0, in1=bhpk_s[:, m * B:(m + 1) * B],
                                op0=AL.mult, op1=AL.add)

                    # ---- attention rows + gh interleaved
                    ghge = Psm.tile([B, 2 * H], BF16, tag="ghge",
                                    name=f"ghge{t}")
                    hn_sb = Psm.tile([B, H], BF16, tag="hn", name=f"hn{t}")
                    scores_sb = Psm.tile([B, N], BF16, tag="scores",
                                         name=f"scores{t}")
                    with tc.tile_pool(name="psB", bufs=1, space="PSUM") as PB, \
                         tc.tile_pool(name="psG", bufs=1, space="PSUM") as PG:
                        sc_ps = [PB.tile([B, SCW], F32, tag=f"sc{c}",
                                         name=f"sc{t}_{c}")
                                 for c in range(NSC)]

                        def gh_chunk(c):
                            ps = PG.tile([B, 512], F32, tag="ghp",
                                         name=f"ghp{t}_{c}")
                            for k in range(KH):
                                nc.tensor.matmul(
                                    ps, hpk[:, k * B:(k + 1) * B],
                                    whhT_s[:, k, c * 512:(c + 1) * 512],
                                    start=(k == 0), stop=(k == KH - 1))
                            if c < 4:
                                nc.vector.scalar_tensor_tensor(
                                    out=ghge[:, c * 512:(c + 1) * 512],
                                    in0=ps, scalar=0.5,
                                    in1=ge_t[:, c * 512:(c + 1) * 512],
                                    op0=AL.mult, op1=AL.add)
                            else:
                                nc.scalar.activation(
                                    hn_sb[:, (c - 4) * 512:(c - 3) * 512],
                                    ps, AF.Copy)

                        ghq = list(range(6))
                        for j in range(KH):
                            xr = Px.tile([128, BN], BF16, tag="x",
                                         name=f"x{t}_{j}")
                            for h2 in range(2):
                                pj = Pstr.tile([128, HBN], BF16, tag="proj",
                                               name=f"pj{t}_{j}_{h2}")
                                nc.sync.dma_start(
                                    pj,
                                    projT_d[j][:, h2 * HBN:(h2 + 1) * HBN])
                                pj3 = pj.rearrange("p (b n) -> p b n", n=N)
                                hqb = hq_sb[:, j * B + 8 * h2:
                                            j * B + 8 * h2 + 8] \
                                    .unsqueeze(2).broadcast_to([128, 8, N])
                                idx = j * 2 + h2
                                eng = (nc.vector if (idx % 8) < 5
                                       else nc.gpsimd)
                                eng.tensor_tensor(out=pj3, in0=pj3, in1=hqb,
                                                  op=AL.add)
                                nc.scalar.activation(
                                    xr[:, h2 * HBN:(h2 + 1) * HBN], pj,
                                    AF.Tanh)
                            for c in range(NSC):
                                nc.tensor.matmul(
                                    sc_ps[c], vrep_s[:, j, :],
                                    xr[:, c * SCW:(c + 1) * SCW],
                                    start=(j == 0), stop=(j == KH - 1))
                            if j < 3:
                                gh_chunk(ghq.pop(0))
                                gh_chunk(ghq.pop(0))

                        # evacuate score chunks into a flat row, then
                        # one reshape DMA into [16, 196]
                        scflat = Psm.tile([1, BN], BF16, tag="scflat",
                                          name=f"scflat{t}")
                        for c in range(NSC):
                            seg = scflat[:, c * SCW:(c + 1) * SCW]
                            if c % 2 == 0:
                                nc.vector.tensor_copy(seg, sc_ps[c][0:1, :])
                            else:
                                nc.scalar.activation(seg, sc_ps[c][0:1, :],
                                                     AF.Copy)
                        nc.sync.dma_start(
                            out=scores_sb,
                            in_=scflat.rearrange("o (b n) -> o b n", n=N))

                    # ---- softmax + w scatter
                    exps = Psm.tile([B, N], BF16, tag="exps", name=f"exps{t}")
                    sumexp = Psm.tile([B, 1], F32, tag="sumexp",
                                      name=f"sumexp{t}")
                    nc.scalar.activation(exps, scores_sb, AF.Exp,
                                         accum_out=sumexp)
                    rec = Psm.tile([B, 1], F32, tag="rec", name=f"rec{t}")
                    nc.vector.reciprocal(rec, sumexp)
                    nc.vector.tensor_scalar(
                        out=w_s, in0=exps, scalar1=rec, scalar2=None,
                        op0=AL.mult)
                    wT_sb = Psm.tile([128, 2 * B], BF16, tag="wT",
                                     name=f"wT{t}", bufs=2)
                    with tc.tile_pool(name="psW", bufs=2, space="PSUM") as PW:
                        wt0 = PW.tile([128, B], BF16, tag="wt0",
                                      name=f"wt0{t}")
                        nc.tensor.transpose(wt0, w_s[:, 0:128], ident16)
                        nc.vector.tensor_copy(wT_sb[:, 0:B], wt0)
                        wt1 = PW.tile([68, B], BF16, tag="wt1",
                                      name=f"wt1{t}")
                        nc.tensor.transpose(wt1, w_s[:, 128:196], ident16)
                        nc.vector.tensor_copy(wT_sb[0:68, B:2 * B], wt1)
                    wv = wblk.rearrange("p (b r) -> p b r", r=33)
                    nc.sync.dma_start(out=wv[:, :, 0:1],
                                      in_=wT_sb[:, 0:B].unsqueeze(2))
                    nc.sync.dma_start(out=wv[0:68, :, 16:17],
                                      in_=wT_sb[0:68, B:2 * B].unsqueeze(2))

                    # ---- ctx
                    ctxs = Psm.tile([B, H], BF16, tag="hqf", name=f"ctxs{t}")
                    ctxT = Psm.tile([128, 128], BF16, tag="ctxT",
                                    name=f"ctxT{t}")
                    with tc.tile_pool(name="psC", bufs=1, space="PSUM") as PC:
                        ctxL = PC.tile([B, 512], F32, tag="ctxL",
                                       name=f"ctxL{t}")
                        ctxR = PC.tile([B, 512], F32, tag="ctxR",
                                       name=f"ctxR{t}")
                        for kb in range(KB):
                            lhs = wblk[:, kb * B:(kb + 1) * B]
                            nc.tensor.matmul(ctxL, lhs, feat_s[:, kb, 0:512],
                                             start=(kb == 0),
                                             stop=(kb == KB - 1))
                            nc.tensor.matmul(ctxR, lhs,
                                             feat_s[:, kb, 512:1024],
                                             start=(kb == 0),
                                             stop=(kb == KB - 1))
                        nc.vector.tensor_copy(ctxs[:, 0:512], ctxL)
                        nc.vector.tensor_copy(ctxs[:, 512:1024], ctxR)
                    with tc.tile_pool(name="psT2", bufs=2,
                                      space="PSUM") as PT2:
                        for m in range(KH):
                            tp2 = PT2.tile([128, B], BF16, tag="tpc",
                                           name=f"tpc{t}_{m}")
                            nc.tensor.transpose(
                                tp2, ctxs[:, m * 128:(m + 1) * 128], ident16)
                            nc.vector.tensor_copy(
                                ctxT[:, m * B:(m + 1) * B], tp2)

                    # ---- gi (Wx streamed) + gate evac
                    srz = Psm.tile([B, 2 * H], BF16, tag="srz",
                                   name=f"srz{t}")
                    nin = Psm.tile([B, H], BF16, tag="nin", name=f"nin{t}")
                    with tc.tile_pool(name="psGI", bufs=1, space="PSUM") as PGi:
                        gps = [PGi.tile([B, 512], F32, tag=f"gi{c}",
                                        name=f"gi{t}_{c}") for c in range(6)]
                        for k in range(KH):
                            wx_k = Pwx.tile([128, G], BF16, tag="wx",
                                            name=f"wx{t}_{k}")
                            nc.sync.dma_start(wx_k, wxT_d[k])
                            for c in range(6):
                                nc.tensor.matmul(
                                    gps[c], ctxT[:, k * B:(k + 1) * B],
                                    wx_k[:, c * 512:(c + 1) * 512],
                                    start=(k == 0), stop=(k == KH - 1))
                        for c in range(4):
                            nc.vector.scalar_tensor_tensor(
                                out=srz[:, c * 512:(c + 1) * 512], in0=gps[c],
                                scalar=0.5,
                                in1=ghge[:, c * 512:(c + 1) * 512],
                                op0=AL.mult, op1=AL.add)
                        for c in range(2):
                            nc.vector.scalar_tensor_tensor(
                                out=nin[:, c * 512:(c + 1) * 512],
                                in0=gps[4 + c], scalar=1.0,
                                in1=ge_t[:, 2 * H + c * 512:
                                         2 * H + (c + 1) * 512],
                                op0=AL.mult, op1=AL.add)

                    # ---- GRU elementwise ([16, *] layout)
                    t_rz = Psm.tile([B, 2 * H], BF16, tag="trz",
                                    name=f"trz{t}")
                    nc.scalar.activation(t_rz, srz, AF.Tanh)
                    r_ = Pgt.tile([B, H], BF16, tag="gt", name=f"r{t}")
                    nc.vector.tensor_scalar(out=r_, in0=t_rz[:, 0:H],
                                            scalar1=0.5, scalar2=0.5,
                                            op0=AL.mult, op1=AL.add)
                    rhn = Pgt.tile([B, H], BF16, tag="gt", name=f"rhn{t}")
                    nc.gpsimd.tensor_tensor(out=rhn, in0=r_, in1=hn_sb,
                                            op=AL.mult)
                    narg = Pgt.tile([B, H], BF16, tag="gt", name=f"narg{t}")
                    nc.vector.tensor_tensor(out=narg, in0=rhn, in1=nin,
                                            op=AL.add)
                    n_ = Pgf.tile([B, H], F32, tag="gf", name=f"n{t}")
                    nc.scalar.activation(n_, narg, AF.Tanh)
                    z_ = Pgt.tile([B, H], BF16, tag="gt", name=f"z{t}")
                    nc.gpsimd.tensor_scalar(out=z_, in0=t_rz[:, H:2 * H],
                                            scalar1=0.5, scalar2=0.5,
                                            op0=AL.mult, op1=AL.add)
                    d_ = Pgf.tile([B, H], F32, tag="gf", name=f"d{t}")
                    nc.vector.tensor_tensor(out=d_, in0=h32, in1=n_,
                                            op=AL.subtract)
                    zd = Pgt.tile([B, H], BF16, tag="gt", name=f"zd{t}")
                    nc.gpsimd.tensor_tensor(out=zd, in0=z_, in1=d_,
                                            op=AL.mult)
                    h32n = P2.tile([B, H], F32, tag="h32", name=f"h32_{t}")
                    nc.vector.tensor_tensor(out=h32n, in0=n_, in1=zd,
                                            op=AL.add)
                    h16f = Pgt.tile([B, H], BF16, tag="gt", name=f"h16f{t}")
                    nc.vector.tensor_copy(h16f, h32n)
                    hpk_n = P2.tile([128, 128], BF16, tag="hpk",
                                    name=f"hpk{t}")
                    with tc.tile_pool(name="psT3", bufs=2,
                                      space="PSUM") as PT3:
                        for m in range(KH):
                            tp3 = PT3.tile([128, B], BF16, tag="tph",
                                           name=f"tph{t}_{m}")
                            nc.tensor.transpose(
                                tp3, h16f[:, m * 128:(m + 1) * 128], ident16)
                            nc.vector.tensor_copy(
                                hpk_n[:, m * B:(m + 1) * B], tp3)
                    nc.sync.dma_start(hsd_d[t], hpk_n)
                    h32, hpk = h32n, hpk_n

            # ---- classifier
            with tc.tile_pool(name="clsw", bufs=1) as Pc, \
                 tc.tile_pool(name="outst", bufs=2) as Po, \
                 tc.tile_pool(name="psE", bufs=2, space="PSUM") as PEp:
                wcls_s = Pc.tile([128, KH, C], BF16)
                hs_cls = Pc.tile([128, T, 128], BF16)
                for k in range(KH):
                    nc.sync.dma_start(wcls_s[:, k, :], wclsT_d[k])
                for t in range(T):
                    nc.sync.dma_start(hs_cls[:, t, :], hsd_d[t])
                for mc in range(CT):
                    cw = 128 if mc < CT - 1 else C - 128 * (CT - 1)
                    ps = PEp.tile([128, TB], F32, tag="cls", name=f"cls{mc}")
                    for k in range(KH):
                        nc.tensor.matmul(
                            ps[0:cw, :],
                            wcls_s[:, k, mc * 128:mc * 128 + cw],
                            hs_cls[:, :, k * B:(k + 1) * B],
                            start=(k == 0), stop=(k == KH - 1))
                    ot = Po.tile([128, TB], F32, tag="ot", name=f"ot{mc}")
                    nc.vector.tensor_copy(ot[0:cw, :], ps[0:cw, :])
                    nc.sync.dma_start(out_d[mc, 0:cw, :], ot[0:cw, :])

    _split_waits(nc)
    return nc


def _get_program():
    if "nc" not in _CACHE:
        _CACHE["nc"] = _build_program()
    return _CACHE["nc"]


def _pack_inputs(cnn_feat, labels, sos, h0, embed_table, W_ih, b_ih, W_hh,
                 b_hh, Wh, bh, Wc, bc, v_w, Wcls):
    """Host-side layout prep. Returns list of per-core input dicts."""
    f32 = np.float32
    cnn_feat = np.asarray(cnn_feat, f32)
    labels = np.asarray(labels)
    W_ih = np.asarray(W_ih, f32)
    We = W_ih[:, :E]                     # [G, E]
    Wx = W_ih[:, E:]                     # [G, H]

    Ball = cnn_feat.shape[0]
    emb = np.asarray(embed_table, f32)[labels]               # [128, 17, E]
    emb_in = np.concatenate(
        [np.broadcast_to(np.asarray(sos, f32), (Ball, 1, E)), emb],
        axis=1)[:, :T]
    geh = emb_in @ We.T + np.asarray(b_ih, f32) + np.asarray(b_hh, f32)
    geh[..., :2 * H] *= 0.5              # pre-halve r,z parts  [128, T, G]

    wcT = np.ascontiguousarray(np.asarray(Wc, f32).T).reshape(KH, 128, H).astype(bf)
    wxT = np.ascontiguousarray(Wx.T).reshape(KH, 128, G).astype(bf)
    whhT = np.ascontiguousarray(np.asarray(W_hh, f32).T).reshape(KH, 128, G).astype(bf)
    whT = np.ascontiguousarray(np.asarray(Wh, f32).T).reshape(KH, 128, H).astype(bf)
    wclsT = np.ascontiguousarray(np.asarray(Wcls, f32).T).reshape(KH, 128, C).astype(bf)
    vrep = np.ascontiguousarray(np.broadcast_to(
        np.asarray(v_w, f32).reshape(KH, 128, 1), (KH, 128, B))).astype(bf)
    h0 = np.asarray(h0, f32)
    h0b = np.ascontiguousarray(np.broadcast_to(h0, (B, H)), f32)
    hpk0 = np.ascontiguousarray(np.broadcast_to(
        h0.reshape(KH, 128, 1), (KH, 128, B)).transpose(1, 0, 2).reshape(128, 128)).astype(bf)
    bh_a = np.asarray(bh, f32)
    bhpk = np.ascontiguousarray(np.broadcast_to(
        bh_a.reshape(KH, 128, 1), (KH, 128, B)).transpose(1, 0, 2).reshape(128, 128)).astype(bf)
    bc_a = np.asarray(bc, f32).reshape(1, H).astype(bf)

    in_maps = []
    for core in range(NCORES):
        b0 = core * B
        fc = cnn_feat[b0:b0 + B]                     # [16, 196, 1024]
        featp = np.zeros((B, 256, H), f32)
        featp[:, :N, :] = fc
        featp = featp.reshape(KB, 128, H).astype(bf)
        featT = np.ascontiguousarray(
            fc.transpose(2, 0, 1).reshape(H, BN)).reshape(KH, 128, BN).astype(bf)
        gepack = np.ascontiguousarray(
            geh[b0:b0 + B].transpose(1, 0, 2)).astype(bf)    # [T, B, G]
        in_maps.append({
            "featp": featp,
            "featT": featT,
            "wcT": wcT,
            "wxT": wxT,
            "whhT": whhT,
            "whT": whT,
            "wclsT": wclsT,
            "vrep": vrep,
            "ge": gepack,
            "h0b": h0b,
            "hpk0": hpk0,
            "bhpk": bhpk,
            "bc": bc_a,
        })
    return in_maps


def kernel(cnn_feat, labels, lens, sos, h0, embed_table, W_ih, b_ih, W_hh,
           b_hh, Wh, bh, Wc, bc, v_w, v_b, Wcls, bcls):
    # v_b shifts all scores uniformly -> softmax-invariant -> dropped.
    nc = _get_program()
    in_maps = _pack_inputs(cnn_feat, labels, sos, h0, embed_table, W_ih, b_ih,
                           W_hh, b_hh, Wh, bh, Wc, bc, v_w, Wcls)
    res = run_bass_kernel_spmd(nc, in_maps, list(range(NCORES)))
    outs = []
    bcls = np.asarray(bcls, np.float32)
    for core in range(NCORES):
        o = np.asarray(res.results[core]["out"], np.float32)  # [CT,128,TB]
        o = o.reshape(CT * 128, T, B)                         # [1024, T, B]
        o = o[:C].transpose(2, 1, 0)                          # [B, T, C]
        outs.append(o)
    full = np.concatenate(outs, axis=0) + bcls                # [128, T, C]
    return np.ascontiguousarray(full, np.float32)


if __name__ == "__main__":
    rng = np.random.default_rng(0)
    s = 0.02
    inputs = dict(
        cnn_feat=rng.standard_normal((128, N, H), dtype=np.float32),
        labels=rng.integers(0, C, (128, 17)).astype(np.int32),
        lens=rng.integers(1, 17, (128,)).astype(np.int32),
        sos=(rng.standard_normal(E) * s).astype(np.float32),
        h0=(rng.standard_normal(H) * s).astype(np.float32),
        embed_table=(rng.standard_normal((C, E)) * s).astype(np.float32),
        W_ih=(rng.standard_normal((G, E + H)) * s).astype(np.float32),
        b_ih=np.zeros(G, np.float32),
        W_hh=(rng.standard_normal((G, H)) * s).astype(np.float32),
        b_hh=np.zeros(G, np.float32),
        Wh=(rng.standard_normal((H, H)) * s).astype(np.float32),
        bh=np.zeros(H, np.float32),
        Wc=(rng.standard_normal((H, H)) * s).astype(np.float32),
        bc=np.zeros(H, np.float32),
        v_w=(rng.standard_normal(H) * s).astype(np.float32),
        v_b=np.zeros((), np.float32),
        Wcls=(rng.standard_normal((C, H)) * s).astype(np.float32),
        bcls=np.zeros(C, np.float32),
    )
    out = kernel(**inputs)
    print("out", out.shape, out.dtype, float(np.abs(out).max()))



# revision 17
# speedup vs baseline: 1.1169x; 1.1169x over previous
"""Trainium2 Bass kernel for nn_DecoderRNN (Bahdanau-attention GRU decoder).

v4: data parallel over batch (128 -> 16 per core, 8 cores), bf16 matmuls.

Schedule vs the original baseline:
  - feat (ctx rhs) and W_hh resident in SBUF; Wh streamed (ring 2); Wx
    streamed CHUNK-major (6 x [128, 8, 512] tiles in gi consumption order)
    through a 3-deep ring, prefetched from step start; proj streamed as
    half-row [128,1568] slots (ring 4).
  - scores: 7x448 PSUM chunks with M=1 lhsT (v as a single column).
  - softmax: transpose unnormalized exp; 1/sum(exp) folded into the ctx
    PSUM evacuation (per-partition tensor_scalar); w scatter into the
    block-diagonal lhsT via 2 DVE strided copies (no DMA).
  - GRU gate chunks reordered (r first, z last, tanh-trick sigmoids so the
    ACT table never swaps) with the z-tail the only serial part; h
    transposed from f32 directly (no bf16 staging copy).
  - ACT carries some PSUM evacuations (Copy shares the ACT table).
"""
import os
import sys

sys.path.insert(0, "/opt/trn_rl_repo")

import numpy as np
import ml_dtypes

import concourse.bass as bass
import concourse.tile as tile
from concourse import mybir
from concourse.bass_utils import run_bass_kernel_spmd
from concourse.masks import make_identity

F32 = mybir.dt.float32
BF16 = mybir.dt.bfloat16
bf = ml_dtypes.bfloat16
AL = mybir.AluOpType
AF = mybir.ActivationFunctionType

NCORES = 8
B = 16            # local batch per core
N = 196           # attention positions
H = 1024          # hidden
E = 512           # embed dim
G = 3 * H         # gate width
T = int(os.environ.get("DECODER_STEPS", "17"))
C = 1000          # classes
BN = B * N        # 3136
HBN = BN // 2     # 1568
KH = 8            # h k-tiles (1024/128)
KB = 32           # padded (b,n) k-tiles (16*256/128)
SCW = 448         # scores chunk width (7 chunks -> 7 PSUM banks)
NSC = 7
SU = 448          # startup chunk width (7 chunks)
NSU = 7
CT = 8            # classifier m-tiles (1000 -> 7*128+104)
TB = T * B
GI_ORDER = [0, 1, 4, 5, 2, 3]   # gi chunk consumption order (r, n, z)

_CACHE = {}


def _split_waits(nc, keep=1):
    """This container's walrus build rejects >1 sem-wait per instruction
    (setupSyncWait: 'Too many sync wait commands'). Hoist all but one wait
    of every instruction onto single-wait NoOps on the same engine, placed
    immediately before it in program order."""
    nfix = 0
    for bb in nc.main_func.blocks:
        il = bb.instructions
        i = 0
        while i < len(il):
            ins = il[i]
            si = getattr(ins, 'sync_info', None)
            if si is not None and len(si.on_wait) > keep:
                waits = list(si.on_wait)
                for w_i, w in enumerate(waits[:-keep]):
                    nop = mybir.InstNoOp(name=f"{ins.name}-ws{w_i}", ins=[],
                                         outs=[])
                    nop.engine = ins.engine
                    nop.sync_info = mybir.SyncInfo(on_wait=[w], on_update=[])
                    il.insert(i, nop)
                    i += 1
                ins.sync_info = mybir.SyncInfo(on_wait=waits[-keep:],
                                               on_update=list(si.on_update))
                nfix += 1
            i += 1
    return nfix


def _build_program():
    nc = bass.Bass()

    featp_d = nc.declare_dram_parameter("featp", [KB, 128, H], BF16, isOutput=False)
    featT_d = nc.declare_dram_parameter("featT", [KH, 128, BN], BF16, isOutput=False)
    wcT_d = nc.declare_dram_parameter("wcT", [KH, 128, H], BF16, isOutput=False)
    whhT_d = nc.declare_dram_parameter("whhT", [KH, 128, G], BF16, isOutput=False)
    whT_d = nc.declare_dram_parameter("whT", [KH, 128, H], BF16, isOutput=False)
    wxc_d = nc.declare_dram_parameter("wxc", [6, 128, KH * 512], BF16, isOutput=False)
    wclsT_d = nc.declare_dram_parameter("wclsT", [KH, 128, C], BF16, isOutput=False)
    vcol_d = nc.declare_dram_parameter("vcol", [KH, 128, 1], BF16, isOutput=False)
    ge_d = nc.declare_dram_parameter("ge", [T, B, G], BF16, isOutput=False)
    h0b_d = nc.declare_dram_parameter("h0b", [B, H], F32, isOutput=False)
    hpk0_d = nc.declare_dram_parameter("hpk0", [128, 128], BF16, isOutput=False)
    bhpk_d = nc.declare_dram_parameter("bhpk", [128, 128], BF16, isOutput=False)
    out_d = nc.declare_dram_parameter("out", [CT, 128, TB], F32, isOutput=True)

    projT_d = nc.dram_tensor("projT", [KH, 128, BN], BF16)
    hsd_d = nc.dram_tensor("hsd", [T, 128, 128], BF16)

    # engine pattern for the 16 broadcast-add halves per step (10 DVE, 6 GP)
    ADD_ENG = [0, 0, 1, 0, 1, 0, 1, 0, 0, 1, 0, 1, 0, 1, 0, 0]

    with tile.TileContext(nc) as tc:
        with tc.tile_pool(name="persist", bufs=1) as P1, \
             tc.tile_pool(name="state", bufs=2) as P2:

            # ---- persistent tensors (loads on the gpsimd queue so the
            # startup featT/wcT stream on sync isn't delayed)
            feat_s = P1.tile([128, KB, H], BF16)
            for kb in range(KB):
                nc.gpsimd.dma_start(feat_s[:, kb, :], featp_d[kb])
            whhT_s = P1.tile([128, KH, G], BF16)
            vcol_s = P1.tile([128, KH, 1], BF16)
            for k in range(KH):
                nc.gpsimd.dma_start(whhT_s[:, k, :], whhT_d[k])
                nc.gpsimd.dma_start(vcol_s[:, k, :], vcol_d[k])
            ident16 = P1.tile([B, B], BF16)
            make_identity(nc, ident16)
            ident16f = P1.tile([B, B], F32)
            make_identity(nc, ident16f)
            bhpk_s = P1.tile([128, 128], BF16)
            nc.gpsimd.dma_start(bhpk_s, bhpk_d[:])
            wblk = P1.tile([128, 33 * B], BF16)
            nc.vector.memset(wblk, 0.0)

            h32 = P2.tile([B, H], F32, tag="h32")
            nc.gpsimd.dma_start(h32, h0b_d[:])
            hpk = P2.tile([128, 128], BF16, tag="hpk")
            nc.gpsimd.dma_start(hpk, hpk0_d[:])

            # ---- startup: cnn_proj = feat @ Wc^T -> projT_d (HBM), no bias
            # (bc folded into bhpk on host)
            with tc.tile_pool(name="wcpool", bufs=1) as Pwc, \
                 tc.tile_pool(name="ftring", bufs=12) as Pft, \
                 tc.tile_pool(name="stage", bufs=4) as Pstage, \
                 tc.tile_pool(name="ps_start", bufs=3, space="PSUM") as PSs:
                wcT_s = Pwc.tile([128, KH, H], BF16)
                for k in range(KH):
                    nc.sync.dma_start(wcT_s[:, k, :], wcT_d[k])
                for cch in range(NSU):
                    sl = slice(cch * SU, (cch + 1) * SU)
                    fts = []
                    for k in range(KH):
                        ft = Pft.tile([128, SU], BF16, tag="ft",
                                      name=f"ft{cch}_{k}")
                        nc.sync.dma_start(ft, featT_d[k][:, sl])
                        fts.append(ft)
                    for m in range(KH):
                        ps = PSs.tile([128, SU], F32, tag="ps",
                                      name=f"ps{cch}_{m}")
                        for k in range(KH):
                            nc.tensor.matmul(
                                ps, wcT_s[:, k, m * 128:(m + 1) * 128], fts[k],
                                start=(k == 0), stop=(k == KH - 1))
                        st = Pstage.tile([128, SU], BF16, tag="st",
                                         name=f"st{cch}_{m}")
                        if m % 2 == 0:
                            nc.vector.tensor_copy(st, ps)
                        else:
                            nc.scalar.activation(st, ps, AF.Copy)
                        nc.sync.dma_start(projT_d[m][:, sl], st)

            # ---- decode steps
            with tc.tile_pool(name="projring", bufs=4) as Pstr, \
                 tc.tile_pool(name="xring", bufs=2) as Px, \
                 tc.tile_pool(name="whtring", bufs=2) as Pwht, \
                 tc.tile_pool(name="wxring", bufs=2) as Pwx, \
                 tc.tile_pool(name="gering", bufs=1) as Pge, \
                 tc.tile_pool(name="small", bufs=1) as Psm, \
                 tc.tile_pool(name="hqsb", bufs=2) as Phq, \
                 tc.tile_pool(name="nf", bufs=2) as Pgf, \
                 tc.tile_pool(name="gt", bufs=2) as Pgt:
                for t in range(T):
                    ge_t = Pge.tile([B, G], BF16, tag="ge", name=f"ge{t}")
                    nc.sync.dma_start(ge_t, ge_d[t])
                    # prefetch first proj half-slots and first 2 wx chunks
                    pslots = []
                    for s_i in range(3):
                        pj = Pstr.tile([128, HBN], BF16, tag="proj",
                                       name=f"pj{t}_{s_i}")
                        nc.sync.dma_start(
                            pj, projT_d[s_i // 2][:, (s_i % 2) * HBN:
                                                  (s_i % 2 + 1) * HBN])
                        pslots.append(pj)
                    wxs = []
                    for ci in range(2):
                        wx = Pwx.tile([128, KH, 512], BF16, tag="wx",
                                      name=f"wx{t}_{ci}")
                        nc.scalar.dma_start(
                            wx.rearrange("p k q -> p (k q)"), wxc_d[ci])
                        wxs.append(wx)

                    # ---- hq = h @ Wh^T (+ bh + bc via bhpk), packed
                    hqf = Psm.tile([B, H], BF16, tag="hqf", name=f"hqf{t}")
                    hq_sb = Phq.tile([128, 128], BF16, tag="hqsb",
                                     name=f"hqsb{t}")
                    with tc.tile_pool(name="psA", bufs=1, space="PSUM") as PA, \
                         tc.tile_pool(name="psT", bufs=2, space="PSUM") as PT:
                        pqs = [PA.tile([B, 512], F32, tag=f"hqp{c}",
                                       name=f"hqp{t}_{c}") for c in range(2)]
                        for k in range(KH):
                            wht_k = Pwht.tile([128, H], BF16, tag="wht",
                                              name=f"wht{t}_{k}")
                            nc.sync.dma_start(wht_k, whT_d[k])
                            for c in range(2):
                                nc.tensor.matmul(
                                    pqs[c], hpk[:, k * B:(k + 1) * B],
                                    wht_k[:, c * 512:(c + 1) * 512],
                                    start=(k == 0), stop=(k == KH - 1))
                        nc.vector.tensor_copy(hqf[:, 0:512], pqs[0])
                        nc.scalar.copy(hqf[:, 512:1024], pqs[1])
                        for m in range(KH):
                            tp = PT.tile([128, B], BF16, tag="tphq",
                                         name=f"tphq{t}_{m}")
                            nc.tensor.transpose(
                                tp, hqf[:, m * 128:(m + 1) * 128], ident16)
                            nc.vector.scalar_tensor_tensor(
                                out=hq_sb[:, m * B:(m + 1) * B], in0=tp,
                                scalar=1.0, in1=bhpk_s[:, m * B:(m + 1) * B],
                                op0=AL.mult, op1=AL.add)

                    # ---- attention: X = tanh(proj + hq) (adds in place on
                    # proj half slots, tanh into xr), scores, gh interleaved
                    ghge = Psm.tile([B, 2 * H], BF16, tag="ghge",
                                    name=f"ghge{t}")
                    hn_sb = Psm.tile([B, H], BF16, tag="hn", name=f"hn{t}")
                    scores_sb = Psm.tile([B, N], BF16, tag="scores",
                                         name=f"scores{t}")
                    scflat = Psm.tile([1, BN], BF16, tag="scflat",
                                      name=f"scflat{t}")
                    with tc.tile_pool(name="psB", bufs=1, space="PSUM") as PB, \
                         tc.tile_pool(name="psG", bufs=1, space="PSUM") as PG:
                        sc_ps = [PB.tile([1, SCW], F32, tag=f"sc{c}",
                                         name=f"sc{t}_{c}")
                                 for c in range(NSC)]

                        def gh_chunk(c):
                            ps = PG.tile([B, 512], F32, tag="ghp",
                                         name=f"ghp{t}_{c}")
                            for k in range(KH):
                                nc.tensor.matmul(
                                    ps, hpk[:, k * B:(k + 1) * B],
                                    whhT_s[:, k, c * 512:(c + 1) * 512],
                                    start=(k == 0), stop=(k == KH - 1))
                            if c < 4:
                                nc.vector.scalar_tensor_tensor(
                                    out=ghge[:, c * 512:(c + 1) * 512],
                                    in0=ps, scalar=0.5,
                                    in1=ge_t[:, c * 512:(c + 1) * 512],
                                    op0=AL.mult, op1=AL.add)
                            else:
                                nc.vector.tensor_copy(
                                    hn_sb[:, (c - 4) * 512:(c - 3) * 512], ps)

                        for j in range(KH):
                            xr = Px.tile([128, BN], BF16, tag="x",
                                         name=f"x{t}_{j}")
                            for h2 in range(2):
                                s_i = 2 * j + h2
                                pj = pslots[s_i]
                                if s_i + 3 < 16:
                                    nj = s_i + 3
                                    pjn = Pstr.tile([128, HBN], BF16,
                                                    tag="proj",
                                                    name=f"pj{t}_{nj}")
                                    nc.sync.dma_start(
                                        pjn,
                                        projT_d[nj // 2][:, (nj % 2) * HBN:
                                                         (nj % 2 + 1) * HBN])
                                    pslots.append(pjn)
                                pj3 = pj.rearrange("p (b n) -> p b n", n=N)
                                hqb = hq_sb[:, j * B + 8 * h2:
                                            j * B + 8 * h2 + 8] \
                                    .unsqueeze(2).broadcast_to([128, 8, N])
                                eng = (nc.vector if ADD_ENG[s_i] == 0
                                       else nc.gpsimd)
                                eng.tensor_tensor(out=pj3, in0=pj3, in1=hqb,
                                                  op=AL.add)
                                nc.scalar.activation(
                                    xr[:, h2 * HBN:(h2 + 1) * HBN], pj,
                                    AF.Tanh)
                            for c in range(NSC):
                                nc.tensor.matmul(
                                    sc_ps[c], vcol_s[:, j, :],
                                    xr[:, c * SCW:(c + 1) * SCW],
                                    start=(j == 0), stop=(j == KH - 1))
                            if j < 6:
                                gh_chunk(j)

                        for c in range(NSC):
                            seg = scflat[:, c * SCW:(c + 1) * SCW]
                            if c % 2 == 0:
                                nc.scalar.copy(seg, sc_ps[c][0:1, :])
                            else:
                                nc.vector.tensor_copy(seg, sc_ps[c][0:1, :])
                        nc.sync.dma_start(
                            out=scores_sb,
                            in_=scflat.rearrange("o (b n) -> o b n", n=N))

                    # ---- softmax (unnormalized; 1/sum folded into ctx evac)
                    exps = Psm.tile([B, N], BF16, tag="exps", name=f"exps{t}")
                    sumexp = Psm.tile([B, 1], F32, tag="sumexp",
                                      name=f"sumexp{t}")
                    nc.scalar.activation(exps, scores_sb, AF.Exp,
                                         accum_out=sumexp)
                    rec = Psm.tile([B, 1], F32, tag="rec", name=f"rec{t}")
                    nc.vector.reciprocal(rec, sumexp)
                    wT_sb = Psm.tile([128, 2 * B], BF16, tag="wT",
                                     name=f"wT{t}")
                    with tc.tile_pool(name="psW", bufs=2, space="PSUM") as PW:
                        wt0 = PW.tile([128, B], BF16, tag="wt0",
                                      name=f"wt0{t}")
                        nc.tensor.transpose(wt0, exps[:, 0:128], ident16)
                        nc.vector.tensor_copy(wT_sb[:, 0:B], wt0)
                        wt1 = PW.tile([68, B], BF16, tag="wt1",
                                      name=f"wt1{t}")
                        nc.tensor.transpose(wt1, exps[:, 128:196], ident16)
                        nc.vector.tensor_copy(wT_sb[0:68, B:2 * B], wt1)
                    wv = wblk.rearrange("p (b r) -> p b r", r=33)
                    nc.vector.tensor_copy(out=wv[:, :, 0:1],
                                          in_=wT_sb[:, 0:B].unsqueeze(2))
                    nc.vector.tensor_copy(out=wv[0:68, :, 16:17],
                                          in_=wT_sb[0:68, B:2 * B]
                                          .unsqueeze(2))

                    # ---- ctx (feat resident; 1/sumexp via tensor_scalar)
                    ctxs = Psm.tile([B, H], BF16, tag="ctxs", name=f"ctxs{t}")
                    ctxT = Psm.tile([128, 128], BF16, tag="ctxT",
                                    name=f"ctxT{t}")
                    with tc.tile_pool(name="psC", bufs=1, space="PSUM") as PC:
                        ctxL = PC.tile([B, 512], F32, tag="ctxL",
                                       name=f"ctxL{t}")
                        ctxR = PC.tile([B, 512], F32, tag="ctxR",
                                       name=f"ctxR{t}")
                        for kb in range(KB):
                            lhs = wblk[:, kb * B:(kb + 1) * B]
                            nc.tensor.matmul(ctxL, lhs, feat_s[:, kb, 0:512],
                                             start=(kb == 0),
                                             stop=(kb == KB - 1))
                            nc.tensor.matmul(ctxR, lhs,
                                             feat_s[:, kb, 512:1024],
                                             start=(kb == 0),
                                             stop=(kb == KB - 1))
                        nc.vector.tensor_scalar(
                            out=ctxs[:, 0:512], in0=ctxL, scalar1=rec,
                            scalar2=None, op0=AL.mult)
                        nc.vector.tensor_scalar(
                            out=ctxs[:, 512:1024], in0=ctxR, scalar1=rec,
                            scalar2=None, op0=AL.mult)
                    with tc.tile_pool(name="psT2", bufs=2,
                                      space="PSUM") as PT2:
                        for m in range(KH):
                            tp2 = PT2.tile([128, B], BF16, tag="tpc",
                                           name=f"tpc{t}_{m}")
                            nc.tensor.transpose(
                                tp2, ctxs[:, m * 128:(m + 1) * 128], ident16)
                            if m % 2 == 0:
                                nc.vector.tensor_copy(
                                    ctxT[:, m * B:(m + 1) * B], tp2)
                            else:
                                nc.scalar.copy(
                                    ctxT[:, m * B:(m + 1) * B], tp2)

                    # ---- gi = ctx @ Wx^T, chunk order r(0,1) n(4,5) z(2,3);
                    # gate elementwise interleaved (tanh-trick sigmoids)
                    tr_ = Pgt.tile([B, H], BF16, tag="gt", name=f"tr{t}")
                    r_ = Pgt.tile([B, H], BF16, tag="gt2", name=f"r{t}")
                    rhn = Pgt.tile([B, H], BF16, tag="gt", name=f"rhn{t}")
                    narg = Pgt.tile([B, H], BF16, tag="gt2", name=f"narg{t}")
                    n_ = Pgf.tile([B, H], F32, tag="gf", name=f"n{t}")
                    d_ = Pgf.tile([B, H], F32, tag="gf", name=f"d{t}")
                    tz_ = Pgt.tile([B, H], BF16, tag="gt", name=f"tz{t}")
                    z_ = Pgt.tile([B, H], BF16, tag="gt2", name=f"z{t}")
                    zd = Pgt.tile([B, H], BF16, tag="gt", name=f"zd{t}")
                    h32n = P2.tile([B, H], F32, tag="h32", name=f"h32_{t}")
                    with tc.tile_pool(name="psGI", bufs=1, space="PSUM") as PGi:
                        for ci, c in enumerate(GI_ORDER):
                            if ci + 2 < 6:
                                wx = Pwx.tile([128, KH, 512], BF16, tag="wx",
                                              name=f"wx{t}_{ci + 2}")
                                nc.scalar.dma_start(
                                    wx.rearrange("p k q -> p (k q)"),
                                    wxc_d[ci + 2])
                                wxs.append(wx)
                            ps = PGi.tile([B, 512], F32, tag=f"gi{c}",
                                          name=f"gi{t}_{c}")
                            for k in range(KH):
                                nc.tensor.matmul(
                                    ps, ctxT[:, k * B:(k + 1) * B],
                                    wxs[ci][:, k, :],
                                    start=(k == 0), stop=(k == KH - 1))
                            if c < 4:
                                sl = slice(c * 512, (c + 1) * 512)
                                nc.vector.scalar_tensor_tensor(
                                    out=ghge[:, sl], in0=ps, scalar=0.5,
                                    in1=ghge[:, sl], op0=AL.mult, op1=AL.add)
                            else:
                                sl = slice(2 * H + (c - 4) * 512,
                                           2 * H + (c - 3) * 512)
                                nc.vector.scalar_tensor_tensor(
                                    out=ge_t[:, sl], in0=ps, scalar=1.0,
                                    in1=ge_t[:, sl], op0=AL.mult, op1=AL.add)
                            if c == 1:
                                nc.scalar.activation(tr_, ghge[:, 0:H],
                                                     AF.Tanh)
                                nc.vector.tensor_scalar(
                                    out=r_, in0=tr_, scalar1=0.5, scalar2=0.5,
                                    op0=AL.mult, op1=AL.add)
                            elif c == 5:
                                nc.vector.tensor_tensor(
                                    out=rhn, in0=r_, in1=hn_sb, op=AL.mult)
                                nc.gpsimd.tensor_tensor(
                                    out=narg, in0=rhn,
                                    in1=ge_t[:, 2 * H:3 * H], op=AL.add)
                                nc.scalar.activation(n_, narg, AF.Tanh)
                                nc.vector.tensor_tensor(
                                    out=d_, in0=h32, in1=n_, op=AL.subtract)
                    nc.scalar.activation(tz_, ghge[:, H:2 * H], AF.Tanh)
                    nc.gpsimd.tensor_scalar(
                        out=z_, in0=tz_, scalar1=0.5, scalar2=0.5,
                        op0=AL.mult, op1=AL.add)
                    nc.gpsimd.tensor_tensor(out=zd, in0=z_, in1=d_,
                                            op=AL.mult)
                    nc.vector.tensor_tensor(out=h32n, in0=n_, in1=zd,
                                            op=AL.add)
                    hpk_n = P2.tile([128, 128], BF16, tag="hpk",
                                    name=f"hpk{t}")
                    with tc.tile_pool(name="psT3", bufs=2,
                                      space="PSUM") as PT3:
                        for m in range(KH):
                            tp3 = PT3.tile([128, B], F32, tag="tph",
                                           name=f"tph{t}_{m}")
                            nc.tensor.transpose(
                                tp3, h32n[:, m * 128:(m + 1) * 128], ident16f)
                            if m % 2 == 0:
                                nc.vector.tensor_copy(
                                    hpk_n[:, m * B:(m + 1) * B], tp3)
                            else:
                                nc.scalar.copy(
                                    hpk_n[:, m * B:(m + 1) * B], tp3)
                    nc.sync.dma_start(hsd_d[t], hpk_n)
                    h32, hpk = h32n, hpk_n

            # ---- classifier
            with tc.tile_pool(name="clsw", bufs=1) as Pc, \
                 tc.tile_pool(name="outst", bufs=2) as Po, \
                 tc.tile_pool(name="psE", bufs=2, space="PSUM") as PEp:
                wcls_s = Pc.tile([128, KH, C], BF16)
                hs_cls = Pc.tile([128, T, 128], BF16)
                for k in range(KH):
                    nc.sync.dma_start(wcls_s[:, k, :], wclsT_d[k])
                for t in range(T):
                    nc.sync.dma_start(hs_cls[:, t, :], hsd_d[t])
                for mc in range(CT):
                    cw = 128 if mc < CT - 1 else C - 128 * (CT - 1)
                    ps = PEp.tile([128, TB], F32, tag="cls", name=f"cls{mc}")
                    for k in range(KH):
                        nc.tensor.matmul(
                            ps[0:cw, :],
                            wcls_s[:, k, mc * 128:mc * 128 + cw],
                            hs_cls[:, :, k * B:(k + 1) * B],
                            start=(k == 0), stop=(k == KH - 1))
                    ot = Po.tile([128, TB], F32, tag="ot", name=f"ot{mc}")
                    nc.vector.tensor_copy(ot[0:cw, :], ps[0:cw, :])
                    nc.sync.dma_start(out_d[mc, 0:cw, :], ot[0:cw, :])

    _split_waits(nc)
    return nc


def _get_program():
    if "nc" not in _CACHE:
        _CACHE["nc"] = _build_program()
    return _CACHE["nc"]


def _pack_inputs(cnn_feat, labels, sos, h0, embed_table, W_ih, b_ih, W_hh,
                 b_hh, Wh, bh, Wc, bc, v_w, Wcls):
    """Host-side layout prep. Returns list of per-core input dicts."""
    f32 = np.float32
    cnn_feat = np.asarray(cnn_feat, f32)
    labels = np.asarray(labels)
    W_ih = np.asarray(W_ih, f32)
    We = W_ih[:, :E]                     # [G, E]
    Wx = W_ih[:, E:]                     # [G, H]

    Ball = cnn_feat.shape[0]
    emb = np.asarray(embed_table, f32)[labels]               # [128, 17, E]
    emb_in = np.concatenate(
        [np.broadcast_to(np.asarray(sos, f32), (Ball, 1, E)), emb],
        axis=1)[:, :T]
    geh = emb_in @ We.T + np.asarray(b_ih, f32) + np.asarray(b_hh, f32)
    geh[..., :2 * H] *= 0.5              # pre-halve r,z parts (tanh trick)

    wcT = np.ascontiguousarray(np.asarray(Wc, f32).T).reshape(KH, 128, H).astype(bf)
    whhT = np.ascontiguousarray(np.asarray(W_hh, f32).T).reshape(KH, 128, G).astype(bf)
    whT = np.ascontiguousarray(np.asarray(Wh, f32).T).reshape(KH, 128, H).astype(bf)
    # Wx^T chunk-major in gi consumption order: [6, 128, KH*512]
    wxt4 = Wx.T.reshape(KH, 128, 6, 512)
    wxc = np.ascontiguousarray(
        wxt4.transpose(2, 1, 0, 3).reshape(6, 128, KH * 512)[GI_ORDER]
    ).astype(bf)
    wclsT = np.ascontiguousarray(np.asarray(Wcls, f32).T).reshape(KH, 128, C).astype(bf)
    vcol = np.ascontiguousarray(
        np.asarray(v_w, f32).reshape(KH, 128, 1)).astype(bf)
    h0 = np.asarray(h0, f32)
    h0b = np.ascontiguousarray(np.broadcast_to(h0, (B, H)), f32)
    hpk0 = np.ascontiguousarray(np.broadcast_to(
        h0.reshape(KH, 128, 1), (KH, 128, B)).transpose(1, 0, 2).reshape(128, 128)).astype(bf)
    bias_hb = np.asarray(bh, f32) + np.asarray(bc, f32)
    bhpk = np.ascontiguousarray(np.broadcast_to(
        bias_hb.reshape(KH, 128, 1), (KH, 128, B)).transpose(1, 0, 2).reshape(128, 128)).astype(bf)

    in_maps = []
    for core in range(NCORES):
        b0 = core * B
        fc = cnn_feat[b0:b0 + B]                     # [16, 196, 1024]
        featp = np.zeros((B, 256, H), f32)
        featp[:, :N, :] = fc
        featp = featp.reshape(KB, 128, H).astype(bf)
        featT = np.ascontiguousarray(
            fc.transpose(2, 0, 1).reshape(H, BN)).reshape(KH, 128, BN).astype(bf)
        gepack = np.ascontiguousarray(
            geh[b0:b0 + B].transpose(1, 0, 2)).astype(bf)    # [T, B, G]
        in_maps.append({
            "featp": featp,
            "featT": featT,
            "wcT": wcT,
            "whhT": whhT,
            "whT": whT,
            "wxc": wxc,
            "wclsT": wclsT,
            "vcol": vcol,
            "ge": gepack,
            "h0b": h0b,
            "hpk0": hpk0,
            "bhpk": bhpk,
        })
    return in_maps


def kernel(cnn_feat, labels, lens, sos, h0, embed_table, W_ih, b_ih, W_hh,
           b_hh, Wh, bh, Wc, bc, v_w, v_b, Wcls, bcls):
    # v_b shifts all scores uniformly -> softmax-invariant -> dropped.
    nc = _get_program()
    in_maps = _pack_inputs(cnn_feat, labels, sos, h0, embed_table, W_ih, b_ih,
                           W_hh, b_hh, Wh, bh, Wc, bc, v_w, Wcls)
    res = run_bass_kernel_spmd(nc, in_maps, list(range(NCORES)))
    outs = []
    bcls = np.asarray(bcls, np.float32)
    for core in range(NCORES):
        o = np.asarray(res.results[core]["out"], np.float32)  # [CT,128,TB]
        o = o.reshape(CT * 128, T, B)                         # [1024, T, B]
        o = o[:C].transpose(2, 1, 0)                          # [B, T, C]
        outs.append(o)
    full = np.concatenate(outs, axis=0) + bcls                # [128, T, C]
    return np.ascontiguousarray(full, np.float32)


if __name__ == "__main__":
    rng = np.random.default_rng(0)
    s = 0.02
    inputs = dict(
        cnn_feat=rng.standard_normal((128, N, H), dtype=np.float32),
        labels=rng.integers(0, C, (128, 17)).astype(np.int32),
        lens=rng.integers(1, 17, (128,)).astype(np.int32),
        sos=(rng.standard_normal(E) * s).astype(np.float32),
        h0=(rng.standard_normal(H) * s).astype(np.float32),
        embed_table=(rng.standard_normal((C, E)) * s).astype(np.float32),
        W_ih=(rng.standard_normal((G, E + H)) * s).astype(np.float32),
        b_ih=np.zeros(G, np.float32),
        W_hh=(rng.standard_normal((G, H)) * s).astype(np.float32),
        b_hh=np.zeros(G, np.float32),
        Wh=(rng.standard_normal((H, H)) * s).astype(np.float32),
        bh=np.zeros(H, np.float32),
        Wc=(rng.standard_normal((H, H)) * s).astype(np.float32),
        bc=np.zeros(H, np.float32),
        v_w=(rng.standard_normal(H) * s).astype(np.float32),
        v_b=np.zeros((), np.float32),
        Wcls=(rng.standard_normal((C, H)) * s).astype(np.float32),
        bcls=np.zeros(C, np.float32),
    )
    out = kernel(**inputs)
    print("out", out.shape, out.dtype, float(np.abs(out).max()))
